# revision 41
# baseline (speedup 1.0000x reference)
"""Trainium2 Bass kernel for nn_EncoderLayer_73315091743398.

The reference module's attention einsums ('hwink,hwijm->hwinm') sum their k/j
indices independently, so the whole attention block collapses to, per
(h,w)-chunk c and head i, over the flat q matrix qf = x@Wq.T + pe viewed as
(8192, 512) in raw (s,h,w) row order:

    u[s]  = sum_d qf[c*512+s, 64i+d]          (segment row sums)
    a     = softmax_s(u)
    v[d]  = sum_s a[s] * qf[c*512+s, 64i+d]
    row   = tile8(v) @ Wfc.T = v @ M,  M[d,:] = sum_b Wfc[:, 64b+d].T

and attn_out viewed (S,H,W,D) has row A[s'] = row_{c=s'//32, i=(s'%32)//4},
independent of (h,w).  Core k owns raw rows [k*1024,(k+1)*1024): these are
exactly attention chunks {2k, 2k+1} AND the residual/FFN rows for
s' in [64k, 64k+64), so the 8 cores run fully independent SPMD programs
(data-parallel over the flat row dimension; no collectives).
"""

import math
import os
import sys
from contextlib import ExitStack

import numpy as np
import ml_dtypes  # noqa: F401  (registers bfloat16)

for _p in ("/opt/trn_rl_repo", "/root/.axon_site/_ro/trn_rl_repo"):
    if os.path.isdir(_p) and _p not in sys.path:
        sys.path.append(_p)

import concourse.bass as bass
import concourse.bacc as bacc
import concourse.mybir as mybir
import concourse.tile as tile
from concourse.bass_utils import run_bass_kernel_spmd

F32 = mybir.dt.float32
F32R = mybir.dt.float32r
BF16 = mybir.dt.bfloat16
AF = mybir.ActivationFunctionType
ALU = mybir.AluOpType
AX = mybir.AxisListType

S, H, W, D = 512, 4, 4, 512
NH, DEP, DFF = 8, 64, 2048
NCORES = 8
R = 1024          # rows per core of the flat (8192, 512) view
EPS = 1e-5

# packed fp32 constant block column offsets
O_EYE, O_ON1, O_B2, O_G1, O_BE1, O_G2, O_BE2, O_B1G = (
    0, 128, 129, 641, 1153, 1665, 2177, 2689)
O_G1C, O_BE1C = 2705, 2709
NCF = 2713
# packed f32r constant block column offsets
O_MST, O_ZER, O_ONR, O_E8 = 0, 512, 528, 529
O_EYR, O_B2R = 1041, 1169
NCR = 1681

_cached = {}


def build_nc():
    """Build the single-core SPMD Bass/Tile program (same program on all 8)."""
    nc = bacc.Bacc("TRN2", debug=False, target_bir_lowering=False)

    xT = nc.dram_tensor("xT", [D, R], F32R, kind="ExternalInput")
    xR = nc.dram_tensor("xR", [R, D], F32, kind="ExternalInput")
    peR = nc.dram_tensor("peR", [R, D], F32, kind="ExternalInput")
    WqT = nc.dram_tensor("WqT", [D, D], F32R, kind="ExternalInput")
    W1T = nc.dram_tensor("W1T", [D, DFF], BF16, kind="ExternalInput")
    W2T = nc.dram_tensor("W2T", [DFF, D], BF16, kind="ExternalInput")
    CF = nc.dram_tensor("CF", [128, NCF], F32, kind="ExternalInput")
    CR = nc.dram_tensor("CR", [128, NCR], F32R, kind="ExternalInput")
    CB = nc.dram_tensor("CB", [128, 128 + D], BF16, kind="ExternalInput")
    out = nc.dram_tensor("out", [R, D], F32, kind="ExternalOutput")

    with ExitStack() as ctx:
        tc = ctx.enter_context(tile.TileContext(nc))
        cst = ctx.enter_context(tc.tile_pool(name="cst", bufs=1))
        xp = ctx.enter_context(tc.tile_pool(name="xp", bufs=1))
        qp = ctx.enter_context(tc.tile_pool(name="qp", bufs=1))
        wk = ctx.enter_context(tc.tile_pool(name="wk", bufs=2))
        ps = ctx.enter_context(tc.tile_pool(name="ps", bufs=1, space="PSUM"))

        # ---- loads, cheapest-needed-first so PE can start early ----
        # xq[i] holds m-pair (2i, 2i+1), columns laid out (dt, mi, c)
        xq = [xp.tile([128, R], F32R, tag=f"dT{i}", name=f"xq{i}")
              for i in range(4)]
        wq_all = cst.tile([128, 4 * D], F32R, tag="wq", name="wq_all")
        nc.sync.dma_start(wq_all[:].rearrange("p (t j) -> p t j", t=4),
                          WqT.rearrange("(t p) j -> p t j", p=128))
        pe_sb = [xp.tile([128, D], F32, tag=f"pe{m}", name=f"pesb{m}")
                 for m in range(8)]
        for i in range(4):
            nc.sync.dma_start(xq[i][:], xT[i * 128:(i + 1) * 128, :])
            nc.sync.dma_start(pe_sb[2 * i][:],
                              peR[2 * i * 128:(2 * i + 1) * 128, :])
            nc.sync.dma_start(pe_sb[2 * i + 1][:],
                              peR[(2 * i + 1) * 128:(2 * i + 2) * 128, :])
        cf = cst.tile([128, NCF], F32, tag="cf", name="cf")
        nc.sync.dma_start(cf[:], CF[:])
        cr = cst.tile([128, NCR], F32R, tag="cr", name="cr")
        nc.sync.dma_start(cr[:], CR[:])
        cfb = cst.tile([128, 128 + D], BF16, tag="cfb", name="cfb")
        nc.sync.dma_start(cfb[:], CB[:])
        w1_all = cst.tile([128, 4 * DFF], BF16, tag="w1", name="w1_all")
        nc.sync.dma_start(w1_all[:].rearrange("p (t j) -> p t j", t=4),
                          W1T.rearrange("(t p) j -> p t j", p=128))
        w2_all = cst.tile([128, 16 * D], BF16, tag="w2", name="w2_all")
        nc.sync.dma_start(w2_all[:].rearrange("p (t j) -> p t j", t=16),
                          W2T.rearrange("(t p) j -> p t j", p=128))

        eye_sb = cf[:, O_EYE:O_EYE + 128]
        on1r = cr[:, O_ONR:O_ONR + 1]
        Mst_sb = cr[:, O_MST:O_MST + D]
        zer8 = cr[:, O_ZER:O_ZER + 8]
        B2_sb = cf[:, O_B2:O_B2 + D]
        G1_sb = cf[:, O_G1:O_G1 + D]
        BE1_sb = cf[:, O_BE1:O_BE1 + D]
        G2_sb = cf[:, O_G2:O_G2 + D]
        BE2_sb = cf[:, O_BE2:O_BE2 + D]
        epsT = cst.tile([128, 1], F32, tag="eps", name="epsT")
        nc.vector.memset(epsT[:], EPS)

        q_sb = [qp.tile([128, D], F32, tag=f"q{m}", name=f"qsb{m}") for m in range(8)]
        o1_sb = [qp.tile([128, D], F32, tag=f"o1{m}", name=f"o1sb{m}") for m in range(8)]
        uT = [qp.tile([8, D], F32, tag=f"uT{c}", name=f"uTsb{c}") for c in range(2)]

        def layernorm(dst, zin, g_t, be_t):
            """dst = LN(zin) * g + be for a 128-row tile (zin SBUF f32)."""
            st6 = wk.tile([128, 6], F32, tag="ls")
            nc.vector.bn_stats(st6[:], zin[:])
            mv = wk.tile([128, 2], F32, tag="lm")
            nc.vector.bn_aggr(mv[:], st6[:])
            mu = mv[:, 0:1]
            sd = wk.tile([128, 1], F32, tag="lsd")
            nc.scalar.activation(sd[:], mv[:, 1:2], AF.Sqrt, bias=epsT[:, :])
            rsd = wk.tile([128, 1], F32, tag="lr")
            nc.vector.reciprocal(rsd[:], sd[:])
            nrm = wk.tile([128, D], F32, tag="ln", bufs=1)
            nc.vector.tensor_scalar(nrm[:], zin[:], mu[:], rsd[:],
                                    op0=ALU.subtract, op1=ALU.mult)
            if be_t is None:
                nc.vector.tensor_mul(dst[:], nrm[:], g_t[:])
            else:
                nc.vector.tensor_mul(nrm[:], nrm[:], g_t[:])
                nc.vector.tensor_add(dst[:], nrm[:], be_t[:])

        def q_stage(m):
            qps = ps.tile([128, D], F32, tag="mmA", bufs=3)
            i, mi = divmod(m, 2)
            for dt in range(4):
                nc.tensor.matmul(
                    qps[:],
                    xq[i][:, dt * 256 + mi * 128:dt * 256 + (mi + 1) * 128],
                    wq_all[:, dt * D:(dt + 1) * D],
                    start=(dt == 0), stop=(dt == 3))
            nc.vector.tensor_add(q_sb[m][:], qps[:], pe_sb[m][:])
            useg = wk.tile([128, 8], F32, tag="useg")
            nc.vector.tensor_reduce(
                useg[:], q_sb[m][:].rearrange("p (h d) -> p h d", h=8),
                axis=AX.X, op=ALU.add)
            utp = ps.tile([8, 128], F32, tag="tp", bufs=2)
            nc.tensor.transpose(utp[:], useg[:], eye_sb)
            c, st = divmod(m, 4)
            nc.vector.tensor_copy(uT[c][:, st * 128:(st + 1) * 128], utp[:])

        def attn_softmax(c):
            mx = wk.tile([8, 1], F32, tag="mx")
            nc.vector.tensor_reduce(mx[:], uT[c][:], axis=AX.X, op=ALU.max)
            nmx = wk.tile([8, 1], F32, tag="nmx")
            nc.vector.tensor_scalar_mul(nmx[:], mx[:], -1.0)
            ex = wk.tile([8, D], F32, tag=f"ex{c}", bufs=1)
            ssum = wk.tile([8, 1], F32, tag="esum")
            nc.scalar.activation(ex[:], uT[c][:], AF.Exp, bias=nmx[:, :],
                                 accum_out=ssum[:])
            rcp = wk.tile([8, 1], F32, tag="ercp")
            nc.vector.reciprocal(rcp[:], ssum[:])
            nc.vector.tensor_scalar_mul(ex[:], ex[:], rcp[:])
            return ex

        def attn_prods(c, a_t):
            aTss = []
            for st in range(4):
                atp = ps.tile([128, 8], F32, tag="tp", bufs=2)
                nc.tensor.transpose(atp[:], a_t[:, st * 128:(st + 1) * 128],
                                    eye_sb[:8, :8])
                aTs = wk.tile([128, 8], F32, tag=f"aT{st}", bufs=1)
                nc.vector.tensor_copy(aTs[:], atp[:])
                aTss.append(aTs)
            return aTss

        def attn_la(c, aTss):
            vm = wk.tile([128, 8], F32R, tag="vm")
            nc.vector.tensor_copy(vm[:], zer8)
            for jt in range(4):
                # wsum[p, i] = sum_s q[s, 128*jt+p] * a_i[s]
                wsum = ps.tile([128, 8], F32, tag="vc", bufs=1)
                for st in range(4):
                    nc.tensor.matmul(
                        wsum[:], q_sb[c * 4 + st][:, jt * 128:(jt + 1) * 128],
                        aTss[st][:], start=(st == 0), stop=(st == 3))
                nc.vector.tensor_copy(vm[0:64, 2 * jt:2 * jt + 1],
                                      wsum[0:64, 2 * jt:2 * jt + 1])
                nc.vector.tensor_copy(vm[64:128, 2 * jt + 1:2 * jt + 2],
                                      wsum[64:128, 2 * jt + 1:2 * jt + 2])
            lap = ps.tile([8, D], F32, tag="vc", bufs=1)
            nc.tensor.matmul(lap[:], vm[:], Mst_sb, start=True, stop=True)
            las = wk.tile([8, D], F32R, tag=f"las{c}", bufs=1)
            nc.vector.tensor_copy(las[:], lap[:])
            return las

        def attn_resid(c, las, jt):
            m = c * 4 + jt
            bcp = ps.tile([128, D], F32, tag="mmB", bufs=2)
            nc.tensor.matmul(bcp[:],
                             cr[0:8, O_E8 + jt * 128:O_E8 + (jt + 1) * 128],
                             las[:], start=True, stop=True)
            xrt = wk.tile([128, D], F32, tag="xr")
            nc.gpsimd.dma_start(xrt[:], xR[m * 128:(m + 1) * 128, :])
            z1 = wk.tile([128, D], F32, tag="z1", bufs=1)
            nc.vector.tensor_add(z1[:], bcp[:], xrt[:])
            layernorm(o1_sb[m], z1, G1_sb, None)

        o1T2 = [xp.tile([128, 2 * R], BF16, tag=f"dTh{i}", name=f"o1Th{i}")
                for i in range(2)]

        def trans_stage(m):
            for dt in range(4):
                tps = ps.tile([128, 128], F32, tag="tp", bufs=2)
                nc.tensor.transpose(tps[:], o1_sb[m][:, dt * 128:(dt + 1) * 128],
                                    eye_sb)
                h, mh = divmod(m, 4)
                nc.vector.tensor_scalar(
                    o1T2[h][:, dt * 512 + mh * 128:dt * 512 + (mh + 1) * 128],
                    tps[:], cf[:, O_BE1C + dt:O_BE1C + dt + 1], None,
                    op0=ALU.add)

        h1store = {}

        def ffn_h1(rq):
            h1s = []
            for ft in range(16):
                p1 = ps.tile([128, 256], F32, tag="mmA", bufs=3)
                for dt in range(4):
                    nc.tensor.matmul(
                        p1[:],
                        w1_all[:, dt * DFF + ft * 128:dt * DFF + (ft + 1) * 128],
                        o1T2[rq // 2][:, dt * 512 + (rq % 2) * 256:
                                      dt * 512 + (rq % 2) * 256 + 256],
                        start=(dt == 0), stop=(dt == 3))
                h1 = wk.tile([128, 256], BF16, tag=f"h1_{ft}", bufs=1)
                nc.scalar.activation(h1[:], p1[:], AF.Relu,
                                     bias=cf[:, O_B1G + ft:O_B1G + ft + 1])
                h1s.append(h1)
            h1store[rq] = h1s

        def ffn_rm(rq):
            h1s = h1store[rq]
            for rm in range(2):
                m = rq * 2 + rm
                p2 = ps.tile([128, D], F32, tag="mmB", bufs=2)
                for ft in range(16):
                    nc.tensor.matmul(
                        p2[:], h1s[ft][:, rm * 128:(rm + 1) * 128],
                        w2_all[:, ft * D:(ft + 1) * D],
                        start=(ft == 0), stop=False)
                nc.tensor.matmul(p2[:], cfb[:, 0:128], cfb[:, 128:128 + D],
                                 start=False, stop=True)
                z2 = wk.tile([128, D], F32, tag="z2", bufs=1)
                nc.vector.tensor_add(z2[:], p2[:], o1_sb[m][:])
                yt = wk.tile([128, D], F32, tag="yt", bufs=1)
                layernorm(yt, z2, G2_sb, BE2_sb)
                nc.sync.dma_start(out[m * 128:(m + 1) * 128, :], yt[:])

        for m in range(4):
            q_stage(m)
        a0 = attn_softmax(0)
        q_stage(4)
        aT0 = attn_prods(0, a0)
        q_stage(5)
        q_stage(6)
        las0 = attn_la(0, aT0)
        q_stage(7)
        for jt in range(4):
            attn_resid(0, las0, jt)
        a1 = attn_softmax(1)
        for m in range(4):
            trans_stage(m)
        aT1 = attn_prods(1, a1)
        las1 = attn_la(1, aT1)
        ffn_h1(0)
        attn_resid(1, las1, 0)
        attn_resid(1, las1, 1)
        ffn_rm(0)
        attn_resid(1, las1, 2)
        attn_resid(1, las1, 3)
        ffn_h1(1)
        ffn_rm(1)
        trans_stage(4)
        trans_stage(5)
        ffn_h1(2)
        trans_stage(6)
        trans_stage(7)
        ffn_rm(2)
        ffn_h1(3)
        ffn_rm(3)

    nc.compile()
    return nc


def _round_f32r(a):
    b = np.ascontiguousarray(a, dtype=np.float32).view(np.uint32)
    out = (b + 0x7FF + ((b >> 12) & 1)) & np.uint32(0xFFFFF000)
    return out.view(np.float32)


def _pe_table():
    pos = np.arange(S, dtype=np.float32)[:, None]
    div = np.exp(np.arange(0, D, 2, dtype=np.float32) * (-math.log(10000.0) / D))
    ang = pos * div
    pe = np.zeros((S, D), np.float32)
    pe[:, 0::2] = np.sin(ang)
    pe[:, 1::2] = np.cos(ang)
    return pe


def make_in_maps(x, Wq, Wfc, W1, b1, W2, b2, g1, be1, g2, be2):
    f32 = lambda a: np.ascontiguousarray(a, dtype=np.float32)
    xf = f32(x).reshape(S * H * W, D)
    pe = _pe_table()
    M = f32(Wfc).reshape(D, NH, DEP).sum(axis=1).T          # (64, 512)
    Mstk = np.concatenate([M, M], axis=0)                   # (128, 512)

    CF = np.zeros((128, NCF), np.float32)
    CF[:, O_EYE:O_EYE + 128] = np.eye(128, dtype=np.float32)
    CF[:, O_ON1] = 1.0
    CF[:, O_B2:O_B2 + D] = np.tile(f32(b2), (128, 1))
    CF[:, O_G1:O_G1 + D] = np.tile(f32(g1), (128, 1))
    CF[:, O_BE1:O_BE1 + D] = np.tile(f32(be1), (128, 1))
    CF[:, O_G2:O_G2 + D] = np.tile(f32(g2), (128, 1))
    CF[:, O_BE2:O_BE2 + D] = np.tile(f32(be2), (128, 1))
    CF[:, O_B1G:O_B1G + 16] = f32(b1).reshape(16, 128).T
    CF[:, O_G1C:O_G1C + 4] = f32(g1).reshape(4, 128).T
    CF[:, O_BE1C:O_BE1C + 4] = f32(be1).reshape(4, 128).T

    CB = np.zeros((128, 128 + D), np.float32)
    CB[:, 0:128] = np.eye(128, dtype=np.float32)
    CB[:, 128:128 + D] = np.tile(f32(b2) + f32(be1), (128, 1))
    CB = np.asarray(CB, dtype="bfloat16")
    CR = np.zeros((128, NCR), np.float32)
    CR[:, O_MST:O_MST + D] = _round_f32r(Mstk)
    CR[:, O_ONR] = 1.0
    CR[:, O_EYR:O_EYR + 128] = np.eye(128, dtype=np.float32)
    CR[:, O_B2R:O_B2R + D] = _round_f32r(np.tile(f32(b2) + f32(be1), (128, 1)))
    for jt in range(4):
        for p in range(128):
            CR[2 * jt + p // 64, O_E8 + jt * 128 + p] = 1.0

    shared = dict(
        WqT=_round_f32r(Wq.T),
        W1T=np.asarray(f32(W1.T), dtype='bfloat16'),
        W2T=np.asarray(f32(W2.T), dtype='bfloat16'),
        CF=CF, CR=CR, CB=CB,
    )
    maps = []
    for k in range(NCORES):
        sl = xf[k * R:(k + 1) * R]
        m = dict(shared)
        slT = _round_f32r(sl.T)
        # xq layout: row-block i = m-pair (2i, 2i+1); columns (dt, mi, c)
        arr = slT.reshape(4, 128, 4, 2, 128)        # (t, p, i, mi, c)
        arr = arr.transpose(2, 1, 0, 3, 4)          # (i, p, t, mi, c)
        m["xT"] = np.ascontiguousarray(arr.reshape(512, 1024))
        m["xR"] = np.ascontiguousarray(sl)
        m["peR"] = np.ascontiguousarray(np.repeat(pe[k * 64:(k + 1) * 64], 16, axis=0))
        maps.append(m)
    return maps


def kernel(x, Wq, Wfc, W1, b1, W2, b2, g1, be1, g2, be2, _results_hook=None,
           _trace=False, _tmpdir=None):
    if "nc" not in _cached:
        _cached["nc"] = build_nc()
    nc = _cached["nc"]
    in_maps = make_in_maps(x, Wq, Wfc, W1, b1, W2, b2, g1, be1, g2, be2)
    res = run_bass_kernel_spmd(nc, in_maps, list(range(NCORES)),
                               trace=_trace, tmpdir=_tmpdir)
    if _results_hook is not None:
        _results_hook(res)
    y = np.concatenate([res.results[k]["out"] for k in range(NCORES)], axis=0)
    return y.reshape(S, H, W, D)


# revision 43
# speedup vs baseline: 1.0149x; 1.0149x over previous
"""Trainium2 Bass kernel for nn_EncoderLayer_73315091743398.

The reference module's attention einsums ('hwink,hwijm->hwinm') sum their k/j
indices independently, so the whole attention block collapses to, per
(h,w)-chunk c and head i, over the flat q matrix qf = x@Wq.T + pe viewed as
(8192, 512) in raw (s,h,w) row order:

    u[s]  = sum_d qf[c*512+s, 64i+d]          (segment row sums)
    a     = softmax_s(u)
    v[d]  = sum_s a[s] * qf[c*512+s, 64i+d]
    row   = tile8(v) @ Wfc.T = v @ M,  M[d,:] = sum_b Wfc[:, 64b+d].T

and attn_out viewed (S,H,W,D) has row A[s'] = row_{c=s'//32, i=(s'%32)//4},
independent of (h,w).  Core k owns raw rows [k*1024,(k+1)*1024): these are
exactly attention chunks {2k, 2k+1} AND the residual/FFN rows for
s' in [64k, 64k+64), so the 8 cores run fully independent SPMD programs
(data-parallel over the flat row dimension; no collectives).
"""

import math
import os
import sys
from contextlib import ExitStack

import numpy as np
import ml_dtypes  # noqa: F401  (registers bfloat16)

for _p in ("/opt/trn_rl_repo", "/root/.axon_site/_ro/trn_rl_repo"):
    if os.path.isdir(_p) and _p not in sys.path:
        sys.path.append(_p)

import concourse.bass as bass
import concourse.bacc as bacc
import concourse.mybir as mybir
import concourse.tile as tile
from concourse.bass_utils import run_bass_kernel_spmd

F32 = mybir.dt.float32
F32R = mybir.dt.float32r
BF16 = mybir.dt.bfloat16
AF = mybir.ActivationFunctionType
ALU = mybir.AluOpType
AX = mybir.AxisListType

S, H, W, D = 512, 4, 4, 512
NH, DEP, DFF = 8, 64, 2048
NCORES = 8
R = 1024          # rows per core of the flat (8192, 512) view
EPS = 1e-5

# packed fp32 constant block column offsets
O_EYE, O_ON1, O_B2, O_G1, O_BE1, O_G2, O_BE2, O_B1G = (
    0, 128, 129, 641, 1153, 1665, 2177, 2689)
O_G1C, O_BE1C = 2705, 2709
NCF = 2713
# packed f32r constant block column offsets
O_MST, O_ZER, O_ONR, O_E8 = 0, 512, 528, 529
O_EYR, O_B2R = 1041, 1169
NCR = 1681

_cached = {}


def build_nc():
    """Build the single-core SPMD Bass/Tile program (same program on all 8)."""
    nc = bacc.Bacc("TRN2", debug=False, target_bir_lowering=False)

    xT = nc.dram_tensor("xT", [D, R], F32R, kind="ExternalInput")
    xR = nc.dram_tensor("xR", [R, D], F32, kind="ExternalInput")
    peR = nc.dram_tensor("peR", [R, D], F32, kind="ExternalInput")
    WqT = nc.dram_tensor("WqT", [D, D], F32R, kind="ExternalInput")
    W1T = nc.dram_tensor("W1T", [D, DFF], BF16, kind="ExternalInput")
    W2T = nc.dram_tensor("W2T", [DFF, D], BF16, kind="ExternalInput")
    CF = nc.dram_tensor("CF", [128, NCF], F32, kind="ExternalInput")
    CR = nc.dram_tensor("CR", [128, NCR], F32R, kind="ExternalInput")
    CB = nc.dram_tensor("CB", [128, 128 + D], BF16, kind="ExternalInput")
    out = nc.dram_tensor("out", [R, D], F32, kind="ExternalOutput")

    with ExitStack() as ctx:
        tc = ctx.enter_context(tile.TileContext(nc))
        cst = ctx.enter_context(tc.tile_pool(name="cst", bufs=1))
        xp = ctx.enter_context(tc.tile_pool(name="xp", bufs=1))
        qp = ctx.enter_context(tc.tile_pool(name="qp", bufs=1))
        wk = ctx.enter_context(tc.tile_pool(name="wk", bufs=2))
        ps = ctx.enter_context(tc.tile_pool(name="ps", bufs=1, space="PSUM"))

        # ---- loads, cheapest-needed-first so PE can start early ----
        # xq[i] holds m-pair (2i, 2i+1), columns laid out (dt, mi, c)
        xq = [xp.tile([128, R], F32R, tag=f"dT{i}", name=f"xq{i}")
              for i in range(4)]
        wq_all = cst.tile([128, 4 * D], F32R, tag="wq", name="wq_all")
        nc.sync.dma_start(wq_all[:].rearrange("p (t j) -> p t j", t=4),
                          WqT.rearrange("(t p) j -> p t j", p=128))
        pe_sb = [xp.tile([128, D], F32, tag=f"pe{m}", name=f"pesb{m}")
                 for m in range(8)]
        for i in range(4):
            nc.sync.dma_start(xq[i][:], xT[i * 128:(i + 1) * 128, :])
            nc.sync.dma_start(pe_sb[2 * i][:],
                              peR[2 * i * 128:(2 * i + 1) * 128, :])
            nc.sync.dma_start(pe_sb[2 * i + 1][:],
                              peR[(2 * i + 1) * 128:(2 * i + 2) * 128, :])
        cf = cst.tile([128, NCF], F32, tag="cf", name="cf")
        nc.sync.dma_start(cf[:], CF[:])
        cr = cst.tile([128, NCR], F32R, tag="cr", name="cr")
        nc.sync.dma_start(cr[:], CR[:])
        cfb = cst.tile([128, 128 + D], BF16, tag="cfb", name="cfb")
        nc.sync.dma_start(cfb[:], CB[:])
        w1_all = cst.tile([128, 4 * DFF], BF16, tag="w1", name="w1_all")
        nc.sync.dma_start(w1_all[:].rearrange("p (t j) -> p t j", t=4),
                          W1T.rearrange("(t p) j -> p t j", p=128))
        w2_all = cst.tile([128, 16 * D], BF16, tag="w2", name="w2_all")
        nc.sync.dma_start(w2_all[:].rearrange("p (t j) -> p t j", t=16),
                          W2T.rearrange("(t p) j -> p t j", p=128))

        eye_sb = cf[:, O_EYE:O_EYE + 128]
        on1r = cr[:, O_ONR:O_ONR + 1]
        Mst_sb = cr[:, O_MST:O_MST + D]
        zer8 = cr[:, O_ZER:O_ZER + 8]
        B2_sb = cf[:, O_B2:O_B2 + D]
        G1_sb = cf[:, O_G1:O_G1 + D]
        BE1_sb = cf[:, O_BE1:O_BE1 + D]
        G2_sb = cf[:, O_G2:O_G2 + D]
        BE2_sb = cf[:, O_BE2:O_BE2 + D]
        epsT = cst.tile([128, 1], F32, tag="eps", name="epsT")
        nc.vector.memset(epsT[:], EPS)

        q_sb = [qp.tile([128, D], F32, tag=f"q{m}", name=f"qsb{m}") for m in range(8)]
        o1_sb = [qp.tile([128, D], F32, tag=f"o1{m}", name=f"o1sb{m}") for m in range(8)]
        uT = [qp.tile([8, D], F32, tag=f"uT{c}", name=f"uTsb{c}") for c in range(2)]

        def layernorm(dst, zin, g_t, be_t):
            """dst = LN(zin) * g + be for a 128-row tile (zin SBUF f32)."""
            st6 = wk.tile([128, 6], F32, tag="ls")
            nc.vector.bn_stats(st6[:], zin[:])
            mv = wk.tile([128, 2], F32, tag="lm")
            nc.vector.bn_aggr(mv[:], st6[:])
            mu = mv[:, 0:1]
            sd = wk.tile([128, 1], F32, tag="lsd")
            nc.scalar.activation(sd[:], mv[:, 1:2], AF.Sqrt, bias=epsT[:, :])
            rsd = wk.tile([128, 1], F32, tag="lr")
            nc.vector.reciprocal(rsd[:], sd[:])
            if be_t is None:
                # raw normalized rows; g applied later off the critical path
                nc.vector.tensor_scalar(dst[:], zin[:], mu[:], rsd[:],
                                        op0=ALU.subtract, op1=ALU.mult)
            else:
                nrm = wk.tile([128, D], F32, tag="ln", bufs=1)
                nc.vector.tensor_scalar(nrm[:], zin[:], mu[:], rsd[:],
                                        op0=ALU.subtract, op1=ALU.mult)
                nc.vector.tensor_mul(nrm[:], nrm[:], g_t[:])
                nc.vector.tensor_add(dst[:], nrm[:], be_t[:])

        def q_stage(m):
            qps = ps.tile([128, D], F32, tag="mmA", bufs=2)
            i, mi = divmod(m, 2)
            for dt in range(4):
                nc.tensor.matmul(
                    qps[:],
                    xq[i][:, dt * 256 + mi * 128:dt * 256 + (mi + 1) * 128],
                    wq_all[:, dt * D:(dt + 1) * D],
                    start=(dt == 0), stop=(dt == 3))
            nc.vector.tensor_add(q_sb[m][:], qps[:], pe_sb[m][:])
            useg = wk.tile([128, 8], F32, tag="useg")
            nc.vector.tensor_reduce(
                useg[:], q_sb[m][:].rearrange("p (h d) -> p h d", h=8),
                axis=AX.X, op=ALU.add)
            utp = ps.tile([8, 128], F32, tag="tp", bufs=2)
            nc.tensor.transpose(utp[:], useg[:], eye_sb)
            c, st = divmod(m, 4)
            nc.vector.tensor_copy(uT[c][:, st * 128:(st + 1) * 128], utp[:])

        def attn_softmax(c):
            mx = wk.tile([8, 1], F32, tag="mx")
            nc.vector.tensor_reduce(mx[:], uT[c][:], axis=AX.X, op=ALU.max)
            nmx = wk.tile([8, 1], F32, tag="nmx")
            nc.vector.tensor_scalar_mul(nmx[:], mx[:], -1.0)
            ex = wk.tile([8, D], F32, tag=f"ex{c}", bufs=1)
            ssum = wk.tile([8, 1], F32, tag="esum")
            nc.scalar.activation(ex[:], uT[c][:], AF.Exp, bias=nmx[:, :],
                                 accum_out=ssum[:])
            rcp = wk.tile([8, 1], F32, tag="ercp")
            nc.vector.reciprocal(rcp[:], ssum[:])
            nc.vector.tensor_scalar_mul(ex[:], ex[:], rcp[:])
            return ex

        def attn_prods(c, a_t):
            aTss = []
            for st in range(4):
                atp = ps.tile([128, 8], F32, tag="tp", bufs=2)
                nc.tensor.transpose(atp[:], a_t[:, st * 128:(st + 1) * 128],
                                    eye_sb[:8, :8])
                aTs = wk.tile([128, 8], F32, tag=f"aT{st}", bufs=1)
                nc.vector.tensor_copy(aTs[:], atp[:])
                aTss.append(aTs)
            return aTss

        def attn_la(c, aTss):
            vm = wk.tile([128, 8], F32R, tag="vm")
            nc.vector.tensor_copy(vm[:], zer8)
            for jt in range(4):
                # wsum[p, i] = sum_s q[s, 128*jt+p] * a_i[s]
                wsum = ps.tile([128, 8], F32, tag="vc", bufs=2)
                for st in range(4):
                    nc.tensor.matmul(
                        wsum[:], q_sb[c * 4 + st][:, jt * 128:(jt + 1) * 128],
                        aTss[st][:], start=(st == 0), stop=(st == 3))
                nc.vector.tensor_copy(vm[0:64, 2 * jt:2 * jt + 1],
                                      wsum[0:64, 2 * jt:2 * jt + 1])
                nc.vector.tensor_copy(vm[64:128, 2 * jt + 1:2 * jt + 2],
                                      wsum[64:128, 2 * jt + 1:2 * jt + 2])
            lap = ps.tile([8, D], F32, tag="vc", bufs=2)
            nc.tensor.matmul(lap[:], vm[:], Mst_sb, start=True, stop=True)
            las = wk.tile([8, D], F32R, tag=f"las{c}", bufs=1)
            nc.vector.tensor_copy(las[:], lap[:])
            return las

        def attn_resid(c, las, jt):
            m = c * 4 + jt
            bcp = ps.tile([128, D], F32, tag="mmB", bufs=2)
            nc.tensor.matmul(bcp[:],
                             cr[0:8, O_E8 + jt * 128:O_E8 + (jt + 1) * 128],
                             las[:], start=True, stop=True)
            xrt = wk.tile([128, D], F32, tag="xr")
            nc.gpsimd.dma_start(xrt[:], xR[m * 128:(m + 1) * 128, :])
            z1 = wk.tile([128, D], F32, tag="z1", bufs=1)
            nc.vector.tensor_add(z1[:], bcp[:], xrt[:])
            layernorm(o1_sb[m], z1, G1_sb, None)

        o1T2 = [xp.tile([128, 2 * R], BF16, tag=f"dTh{i}", name=f"o1Th{i}")
                for i in range(2)]

        def trans_stage(m):
            for dt in range(4):
                tps = ps.tile([128, 128], F32, tag="tp", bufs=2)
                nc.tensor.transpose(tps[:], o1_sb[m][:, dt * 128:(dt + 1) * 128],
                                    eye_sb)
                h, mh = divmod(m, 4)
                nc.vector.tensor_scalar(
                    o1T2[h][:, dt * 512 + mh * 128:dt * 512 + (mh + 1) * 128],
                    tps[:], cf[:, O_G1C + dt:O_G1C + dt + 1],
                    cf[:, O_BE1C + dt:O_BE1C + dt + 1],
                    op0=ALU.mult, op1=ALU.add)
            nc.vector.tensor_mul(o1_sb[m][:], o1_sb[m][:], G1_sb[:])

        h1store = {}

        def ffn_h1(rq):
            h1s = []
            for ft in range(16):
                p1 = ps.tile([128, 256], F32, tag="mmA", bufs=2)
                for dt in range(4):
                    nc.tensor.matmul(
                        p1[:],
                        w1_all[:, dt * DFF + ft * 128:dt * DFF + (ft + 1) * 128],
                        o1T2[rq // 2][:, dt * 512 + (rq % 2) * 256:
                                      dt * 512 + (rq % 2) * 256 + 256],
                        start=(dt == 0), stop=(dt == 3))
                h1 = wk.tile([128, 256], BF16, tag=f"h1_{ft}", bufs=1)
                nc.scalar.activation(h1[:], p1[:], AF.Relu,
                                     bias=cf[:, O_B1G + ft:O_B1G + ft + 1])
                h1s.append(h1)
            h1store[rq] = h1s

        def ffn_rm(rq):
            h1s = h1store[rq]
            for rm in range(2):
                m = rq * 2 + rm
                p2 = ps.tile([128, D], F32, tag="mmB", bufs=2)
                for ft in range(16):
                    nc.tensor.matmul(
                        p2[:], h1s[ft][:, rm * 128:(rm + 1) * 128],
                        w2_all[:, ft * D:(ft + 1) * D],
                        start=(ft == 0), stop=False)
                nc.tensor.matmul(p2[:], cfb[:, 0:128], cfb[:, 128:128 + D],
                                 start=False, stop=True)
                z2 = wk.tile([128, D], F32, tag="z2", bufs=1)
                nc.vector.tensor_add(z2[:], p2[:], o1_sb[m][:])
                yt = wk.tile([128, D], F32, tag="yt", bufs=1)
                layernorm(yt, z2, G2_sb, BE2_sb)
                nc.sync.dma_start(out[m * 128:(m + 1) * 128, :], yt[:])

        for m in range(4):
            q_stage(m)
        a0 = attn_softmax(0)
        q_stage(4)
        aT0 = attn_prods(0, a0)
        q_stage(5)
        q_stage(6)
        las0 = attn_la(0, aT0)
        q_stage(7)
        for jt in range(4):
            attn_resid(0, las0, jt)
        a1 = attn_softmax(1)
        for m in range(4):
            trans_stage(m)
        aT1 = attn_prods(1, a1)
        las1 = attn_la(1, aT1)
        ffn_h1(0)
        attn_resid(1, las1, 0)
        attn_resid(1, las1, 1)
        ffn_rm(0)
        attn_resid(1, las1, 2)
        attn_resid(1, las1, 3)
        ffn_h1(1)
        ffn_rm(1)
        trans_stage(4)
        trans_stage(5)
        ffn_h1(2)
        trans_stage(6)
        trans_stage(7)
        ffn_rm(2)
        ffn_h1(3)
        ffn_rm(3)

    nc.compile()
    return nc


def _round_f32r(a):
    b = np.ascontiguousarray(a, dtype=np.float32).view(np.uint32)
    out = (b + 0x7FF + ((b >> 12) & 1)) & np.uint32(0xFFFFF000)
    return out.view(np.float32)


def _pe_table():
    pos = np.arange(S, dtype=np.float32)[:, None]
    div = np.exp(np.arange(0, D, 2, dtype=np.float32) * (-math.log(10000.0) / D))
    ang = pos * div
    pe = np.zeros((S, D), np.float32)
    pe[:, 0::2] = np.sin(ang)
    pe[:, 1::2] = np.cos(ang)
    return pe


def make_in_maps(x, Wq, Wfc, W1, b1, W2, b2, g1, be1, g2, be2):
    f32 = lambda a: np.ascontiguousarray(a, dtype=np.float32)
    xf = f32(x).reshape(S * H * W, D)
    pe = _pe_table()
    M = f32(Wfc).reshape(D, NH, DEP).sum(axis=1).T          # (64, 512)
    Mstk = np.concatenate([M, M], axis=0)                   # (128, 512)

    CF = np.zeros((128, NCF), np.float32)
    CF[:, O_EYE:O_EYE + 128] = np.eye(128, dtype=np.float32)
    CF[:, O_ON1] = 1.0
    CF[:, O_B2:O_B2 + D] = np.tile(f32(b2), (128, 1))
    CF[:, O_G1:O_G1 + D] = np.tile(f32(g1), (128, 1))
    CF[:, O_BE1:O_BE1 + D] = np.tile(f32(be1), (128, 1))
    CF[:, O_G2:O_G2 + D] = np.tile(f32(g2), (128, 1))
    CF[:, O_BE2:O_BE2 + D] = np.tile(f32(be2), (128, 1))
    CF[:, O_B1G:O_B1G + 16] = f32(b1).reshape(16, 128).T
    CF[:, O_G1C:O_G1C + 4] = f32(g1).reshape(4, 128).T
    CF[:, O_BE1C:O_BE1C + 4] = f32(be1).reshape(4, 128).T

    CB = np.zeros((128, 128 + D), np.float32)
    CB[:, 0:128] = np.eye(128, dtype=np.float32)
    CB[:, 128:128 + D] = np.tile(f32(b2) + f32(be1), (128, 1))
    CB = np.asarray(CB, dtype="bfloat16")
    CR = np.zeros((128, NCR), np.float32)
    CR[:, O_MST:O_MST + D] = _round_f32r(Mstk)
    CR[:, O_ONR] = 1.0
    CR[:, O_EYR:O_EYR + 128] = np.eye(128, dtype=np.float32)
    CR[:, O_B2R:O_B2R + D] = _round_f32r(np.tile(f32(b2) + f32(be1), (128, 1)))
    for jt in range(4):
        for p in range(128):
            CR[2 * jt + p // 64, O_E8 + jt * 128 + p] = 1.0

    shared = dict(
        WqT=_round_f32r(Wq.T),
        W1T=np.asarray(f32(W1.T), dtype='bfloat16'),
        W2T=np.asarray(f32(W2.T), dtype='bfloat16'),
        CF=CF, CR=CR, CB=CB,
    )
    maps = []
    for k in range(NCORES):
        sl = xf[k * R:(k + 1) * R]
        m = dict(shared)
        slT = _round_f32r(sl.T)
        # xq layout: row-block i = m-pair (2i, 2i+1); columns (dt, mi, c)
        arr = slT.reshape(4, 128, 4, 2, 128)        # (t, p, i, mi, c)
        arr = arr.transpose(2, 1, 0, 3, 4)          # (i, p, t, mi, c)
        m["xT"] = np.ascontiguousarray(arr.reshape(512, 1024))
        m["xR"] = np.ascontiguousarray(sl)
        m["peR"] = np.ascontiguousarray(np.repeat(pe[k * 64:(k + 1) * 64], 16, axis=0))
        maps.append(m)
    return maps


def kernel(x, Wq, Wfc, W1, b1, W2, b2, g1, be1, g2, be2, _results_hook=None,
           _trace=False, _tmpdir=None):
    if "nc" not in _cached:
        _cached["nc"] = build_nc()
    nc = _cached["nc"]
    in_maps = make_in_maps(x, Wq, Wfc, W1, b1, W2, b2, g1, be1, g2, be2)
    res = run_bass_kernel_spmd(nc, in_maps, list(range(NCORES)),
                               trace=_trace, tmpdir=_tmpdir)
    if _results_hook is not None:
        _results_hook(res)
    y = np.concatenate([res.results[k]["out"] for k in range(NCORES)], axis=0)
    return y.reshape(S, H, W, D)
